# revision 4
# baseline (speedup 1.0000x reference)
"""HKLinear (moe_routing) Trainium2 kernel — 8-core SPMD, data-parallel over
tokens, fp8-e4m3 DoubleRow matmuls with split-precision error correction.

Math (reference):
    x = input.reshape(n, in_f)                       n=8192, in_f=4096
    sm = softmax((x @ centroids.T) / T)              [n, 64], T=0.1
    hits = sm > 0.01
    query_sel = any(hits, axis=1)   -> provably ALL TRUE (max softmax >= 1/64 > 0.01)
    cluster_sel = any(hits, axis=0)                  [64]  (global over ALL tokens)
    row_sel = cluster_sel[assignments]               [out_f]
    out = (x @ W.T + b) * (query_sel & row_sel)      [n, out_f]

Numerics: x = x_hi + x_lo (both e4m3); W*64 = W_hi + W_lo (both e4m3, same
scale).  out ~= (x_hi+x_lo) @ W_hi.T + x_hi @ W_lo[:, :GW*256].T, all matmuls
run at fp8 DoubleRow rate (2 k-rows per cycle).  Host-measured rel err vs the
f32 reference: 1.33e-2 with GW=12 of 16 ksteps corrected (gate is 2e-2).

Schedule: W is the stationary operand (streamed from HBM once); x_hi/x_lo are
SBUF-resident and stream through the PE as the moving operand.  The routing
softmax/threshold runs first on the PE (fp8, margins are ~4.6 in logit space
so fp8 is exact for the mask), the [64] cluster margin is AllReduce(max)'d
across the 8 cores while the main matmul streams; early feature-chunks park
their unmasked psum in SBUF and get masked once the collective lands.
Redundant LDWEIGHTS (same stationary reused by consecutive matmuls) are
deduped post-legalization so the DoubleRow weight-load port is not the
bottleneck.
"""

import numpy as np
import ml_dtypes
from contextlib import contextmanager

N_CORES = 8
IN_F = 4096
OUT_F = 4096
N_CLUSTERS = 64
THRESHOLD = 0.01
TEMPERATURE = 0.1
N_TOKENS = 8192
TOK_PER_CORE = N_TOKENS // N_CORES   # 1024

KSTEPS = IN_F // 256                 # 16 DoubleRow k-steps (256 k each)
NT = OUT_F // 128                    # 32 out-feature tiles
GW = 12                              # ksteps with W_lo correction (of 16)
MASK_AFTER = 9                       # emit mask matmuls after this many chunks
W_SCALE = 64.0                       # W pre-scale into e4m3 range
CT_SCALE = 16.0                      # centroid pre-scale
EXP_SHIFT = -30.0                    # softmax-invariant shift

E4 = ml_dtypes.float8_e4m3
BF16 = ml_dtypes.bfloat16


def _install_ldw_dedup():
    import concourse.tile as tile
    import concourse.mybir as mybir
    if getattr(tile, "_ldw_dedup_installed", False):
        return
    orig_legalize = tile.tile_legalize

    def ldw_key(i):
        return (str(i.ins[0]), str(i.perf_mode), str(i.tile_position),
                str(i.tile_size), str(i.is_transpose))

    def dedup_legalize(ordered, nc):
        out = orig_legalize(ordered, nc)
        for bb, insts in out.items():
            new, last_key, pending = [], None, None
            for i in insts:
                if getattr(i, "engine", None) != mybir.EngineType.PE:
                    new.append(i)
                    continue
                nm = type(i).__name__
                if nm == "InstLdweights":
                    k = ldw_key(i)
                    if k == last_key:
                        pending = i
                        continue
                    last_key = k
                    new.append(i)
                elif nm == "InstMatmult":
                    if pending is not None:
                        i.merge_dependencies_from(pending)
                        pending = None
                    new.append(i)
                else:
                    last_key, pending = None, None
                    new.append(i)
            out[bb] = new
        return out

    tile.tile_legalize = dedup_legalize
    tile._ldw_dedup_installed = True


def _build_bass():
    _install_ldw_dedup()
    import concourse.bass as bass
    import concourse.mybir as mybir
    import concourse.tile as tile
    from concourse import bacc
    from concourse.bass import ds

    f32 = mybir.dt.float32
    bf16 = mybir.dt.bfloat16
    fp8 = mybir.dt.float8e4
    DR = mybir.MatmulPerfMode.DoubleRow

    nc = bacc.Bacc("TRN2", target_bir_lowering=False, debug=False,
                   num_devices=N_CORES)

    # ---- DRAM I/O ----
    xh_d = nc.dram_tensor("xh", [128, KSTEPS, 2, TOK_PER_CORE], fp8, kind="ExternalInput")
    xl_d = nc.dram_tensor("xl", [128, KSTEPS, 2, TOK_PER_CORE], fp8, kind="ExternalInput")
    wh_d = nc.dram_tensor("wh", [NT, 128, KSTEPS, 2, 128], fp8, kind="ExternalInput")
    wl_d = nc.dram_tensor("wl", [NT, 128, GW, 2, 128], fp8, kind="ExternalInput")
    ct_d = nc.dram_tensor("ct", [128, KSTEPS, 2, N_CLUSTERS], fp8, kind="ExternalInput")
    ac_d = nc.dram_tensor("ac", [N_CLUSTERS, NT, 128], bf16, kind="ExternalInput")
    bc_d = nc.dram_tensor("bc", [128, NT], f32, kind="ExternalInput")
    out_d = nc.dram_tensor("out", [NT, 2, 128, 512], f32, kind="ExternalOutput")

    with tile.TileContext(nc) as tc:
        with (
            tc.tile_pool(name="resident", bufs=1) as resident,
            tc.tile_pool(name="wpool", bufs=3) as wpool,
            tc.tile_pool(name="stage", bufs=2 * MASK_AFTER + 2) as stage,
            tc.tile_pool(name="opool", bufs=4) as opool,
            tc.tile_pool(name="route_sb", bufs=1) as route_sb,
            tc.tile_pool(name="psum_main", bufs=4, space="PSUM") as psum_main,
            tc.tile_pool(name="psum_l", bufs=2, space="PSUM") as psum_lp,
            tc.tile_pool(name="psum_scr", bufs=1, space="PSUM") as psum_scr,
            tc.tile_pool(name="cc_dram", bufs=1, space="DRAM") as cc_dram,
        ):
            # ---- resident loads; x chunked per kstep so routing starts early
            xh_sb = resident.tile([128, KSTEPS, 2, TOK_PER_CORE], fp8)
            xl_sb = resident.tile([128, KSTEPS, 2, TOK_PER_CORE], fp8)
            for ks in range(KSTEPS):
                nc.sync.dma_start(xh_sb[:, ks], xh_d[:, ks])
            for ks in range(KSTEPS):
                nc.sync.dma_start(xl_sb[:, ks], xl_d[:, ks])
            ct_sb = resident.tile([128, KSTEPS, 2, N_CLUSTERS], fp8)
            nc.sync.dma_start(ct_sb[:], ct_d[:])
            a_sb = resident.tile([N_CLUSTERS, NT, 128], bf16)
            nc.sync.dma_start(a_sb[:], ac_d[:])
            bc_sb = resident.tile([128, NT], f32)
            nc.sync.dma_start(bc_sb[:], bc_d[:])

            shift_col = route_sb.tile([N_CLUSTERS, 1], f32)
            nc.vector.memset(shift_col[:], EXP_SHIFT)
            ones_c = route_sb.tile([N_CLUSTERS, 1], bf16)
            nc.vector.memset(ones_c[:], 1.0)
            ones_r = route_sb.tile([1, N_CLUSTERS], bf16)
            nc.vector.memset(ones_r[:], 1.0)

            # persistent mask tiles (written mid-program, after the CC)
            ms_sb = route_sb.tile([128, NT], f32)     # mask * 2^-6 (W descale)
            bmask_sb = route_sb.tile([128, NT], f32)  # mask * bias

            # ---- routing: logits on clusters x tokens (fp8 DoubleRow) ----
            cmax_h = []
            for h in range(2):
                psum_l = psum_lp.tile([N_CLUSTERS, 512], f32, tag="psum_l")
                for ks in range(KSTEPS):
                    nc.tensor.matmul(
                        psum_l[:],
                        ct_sb[:, ks],                                  # [128,2,64]
                        xh_sb[:, ks, :, ds(h * 512, 512)],             # [128,2,512]
                        start=(ks == 0), stop=(ks == KSTEPS - 1),
                        perf_mode=DR,
                    )
                # e = exp(l/CT_SCALE - 30), f32 for the compare, bf16 for the sum
                e_f = route_sb.tile([N_CLUSTERS, 512], f32, tag="e_f", bufs=2)
                nc.scalar.activation(e_f[:], psum_l[:],
                                     mybir.ActivationFunctionType.Exp,
                                     bias=shift_col[:], scale=1.0 / CT_SCALE)
                e_b = route_sb.tile([N_CLUSTERS, 512], bf16, tag="e_b", bufs=2)
                nc.scalar.activation(e_b[:], psum_l[:],
                                     mybir.ActivationFunctionType.Exp,
                                     bias=shift_col[:], scale=1.0 / CT_SCALE)
                # S[t] = sum_c e  (PE ones-reduction), then thr*S back on 64 parts
                psum_s = psum_scr.tile([128, 512], f32, tag="scr")
                nc.tensor.matmul(psum_s[ds(0, 1), :], ones_c[:], e_b[:],
                                 start=True, stop=True)
                s_b = route_sb.tile([1, 512], bf16, tag="s_b", bufs=2)
                nc.scalar.activation(s_b[:], psum_s[ds(0, 1), :],
                                     mybir.ActivationFunctionType.Copy,
                                     scale=THRESHOLD)
                psum_b = psum_scr.tile([128, 512], f32, tag="scr")
                nc.tensor.matmul(psum_b[ds(0, N_CLUSTERS), :], ones_r[:], s_b[:],
                                 start=True, stop=True)
                # margin d = e - thr*S ; cluster max over this half's tokens
                d_sb = route_sb.tile([N_CLUSTERS, 512], f32, tag="d_sb", bufs=2)
                nc.vector.tensor_tensor(d_sb[:], e_f[:], psum_b[ds(0, N_CLUSTERS), :],
                                        op=mybir.AluOpType.subtract)
                cm = route_sb.tile([N_CLUSTERS, 1], f32, tag="cm", bufs=2)
                nc.vector.reduce_max(cm[:], d_sb[:], axis=mybir.AxisListType.X)
                cmax_h.append(cm)

            cmax = route_sb.tile([N_CLUSTERS, 1], f32)
            nc.vector.tensor_tensor(cmax[:], cmax_h[0][:], cmax_h[1][:],
                                    op=mybir.AluOpType.max)

            # ---- AllReduce(max) of the [64,1] margins across 8 cores ----
            cc_in = cc_dram.tile([N_CLUSTERS, 1], f32)
            cc_out = cc_dram.tile([N_CLUSTERS, 1], f32, addr_space="Shared")
            nc.gpsimd.dma_start(cc_in[:], cmax[:])
            nc.gpsimd.collective_compute(
                "AllReduce", mybir.AluOpType.max,
                replica_groups=[list(range(N_CORES))],
                ins=[cc_in.opt()], outs=[cc_out.opt()],
            )
            cmax_red = route_sb.tile([N_CLUSTERS, 1], f32)
            nc.gpsimd.dma_start(cmax_red[:], cc_out[:])
            sel_f = route_sb.tile([N_CLUSTERS, 1], f32)
            nc.vector.tensor_scalar(sel_f[:], cmax_red[:], 0.0, None,
                                    op0=mybir.AluOpType.is_gt)
            sel_bf = route_sb.tile([N_CLUSTERS, 1], bf16)
            nc.vector.tensor_copy(sel_bf[:], sel_f[:])

            # ---- main matmul: out[feat, tok] = W_hi.T@(x_hi+x_lo) + W_lo.T@x_hi
            def emit_mm_block(n):
                wh_sb = wpool.tile([128, KSTEPS, 2, 128], fp8, tag="wh_sb")
                nc.sync.dma_start(wh_sb[:], wh_d[n])
                wl_sb = wpool.tile([128, GW, 2, 128], fp8, tag="wl_sb")
                nc.sync.dma_start(wl_sb[:], wl_d[n])
                pd = [psum_main.tile([128, 512], f32, tag="psum_d",
                                     name=f"psum_d_{n}_{h}") for h in range(2)]
                for ks in range(KSTEPS):
                    last = (ks == KSTEPS - 1)
                    # both wh uses adjacent so the reloaded LDWEIGHTS dedupes
                    for h in range(2):
                        rh = ds(h * 512, 512)
                        nc.tensor.matmul(pd[h][:], wh_sb[:, ks], xh_sb[:, ks, :, rh],
                                         start=(ks == 0), stop=False, perf_mode=DR)
                    for h in range(2):
                        rh = ds(h * 512, 512)
                        nc.tensor.matmul(pd[h][:], wh_sb[:, ks], xl_sb[:, ks, :, rh],
                                         start=False, stop=(last and ks >= GW),
                                         perf_mode=DR)
                    if ks < GW:
                        for h in range(2):
                            rh = ds(h * 512, 512)
                            nc.tensor.matmul(pd[h][:], wl_sb[:, ks], xh_sb[:, ks, :, rh],
                                             start=False, stop=(last and ks < GW),
                                             perf_mode=DR)
                return pd

            def emit_epilogue(n, h, src):
                # out = psum * (mask * 2^-6) + bias * mask, per-partition scalars
                o_sb = opool.tile([128, 512], f32, tag="o_sb")
                nc.vector.tensor_scalar(
                    o_sb[:], src,
                    ms_sb[:, ds(n, 1)], bmask_sb[:, ds(n, 1)],
                    op0=mybir.AluOpType.mult, op1=mybir.AluOpType.add,
                )
                nc.sync.dma_start(out_d[n, h], o_sb[:])

            # early chunks: park unmasked psum in SBUF (frees banks), mask later
            parked = []
            for n in range(MASK_AFTER):
                pd = emit_mm_block(n)
                st = []
                for h in range(2):
                    s_t = stage.tile([128, 512], f32, tag="stage")
                    nc.scalar.activation(s_t[:], pd[h][:],
                                         mybir.ActivationFunctionType.Copy)
                    st.append(s_t)
                parked.append(st)

            # ---- row mask from one-hot assignment columns (PE gather) ----
            psum_m = psum_scr.tile([128, 512], f32, tag="scr")
            for n in range(NT):
                nc.tensor.matmul(psum_m[:, ds(n, 1)], a_sb[:, n, :], sel_bf[:],
                                 start=True, stop=True)
            nc.scalar.activation(ms_sb[:], psum_m[:, ds(0, NT)],
                                 mybir.ActivationFunctionType.Copy,
                                 scale=1.0 / W_SCALE)
            m1_sb = route_sb.tile([128, NT], f32)
            nc.scalar.activation(m1_sb[:], psum_m[:, ds(0, NT)],
                                 mybir.ActivationFunctionType.Copy)
            nc.vector.tensor_tensor(bmask_sb[:], m1_sb[:], bc_sb[:],
                                    op=mybir.AluOpType.mult)

            for n in range(MASK_AFTER):
                for h in range(2):
                    emit_epilogue(n, h, parked[n][h][:])

            for n in range(MASK_AFTER, NT):
                pd = emit_mm_block(n)
                for h in range(2):
                    emit_epilogue(n, h, pd[h][:])

    nc.compile()
    return nc


_NC_CACHE = None


def _get_nc():
    global _NC_CACHE
    if _NC_CACHE is None:
        _NC_CACHE = _build_bass()
    return _NC_CACHE


def _pack_x_dr(xs):
    # xs [tok, 4096] f32 -> hi/lo DR packs [128, KSTEPS, 2, tok] fp8
    hi = xs.astype(E4)
    lo = (xs - hi.astype(np.float32)).astype(E4)

    def pack(a):
        # a [tok, k] -> [p, ks, i, tok] with k = ks*256 + i*128 + p
        return np.ascontiguousarray(
            a.T.reshape(KSTEPS, 2, 128, a.shape[0]).transpose(2, 0, 1, 3))
    return pack(hi), pack(lo)


def _prep_in_maps(input, weight, bias, centroids, assignments):
    x = np.ascontiguousarray(np.asarray(input, dtype=np.float32).reshape(N_TOKENS, IN_F))
    w = np.asarray(weight, dtype=np.float32)
    b = np.asarray(bias, dtype=np.float32)
    c = np.asarray(centroids, dtype=np.float32)
    a = np.asarray(assignments)

    ws = w * W_SCALE
    w_hi = ws.astype(E4)
    w_lo = (ws - w_hi.astype(np.float32)).astype(E4)

    def pack_w(m, ksteps):
        # m [out, in] fp8 -> [NT, 128p, ksteps, 2, 128f]
        return np.ascontiguousarray(
            m.reshape(NT, 128, ksteps, 2, 128).transpose(0, 4, 2, 3, 1))
    wh = pack_w(w_hi, KSTEPS)
    wl = pack_w(w_lo[:, :GW * 256], GW)

    cs = (c * (CT_SCALE / TEMPERATURE)).astype(E4)
    ct = np.ascontiguousarray(
        cs.T.reshape(KSTEPS, 2, 128, N_CLUSTERS).transpose(2, 0, 1, 3))

    ac = (a[None, :] == np.arange(N_CLUSTERS, dtype=a.dtype)[:, None])
    ac = np.ascontiguousarray(ac.reshape(N_CLUSTERS, NT, 128)).astype(BF16)
    bc = np.ascontiguousarray(b.reshape(NT, 128).T).astype(np.float32)

    in_maps = []
    for core in range(N_CORES):
        xs = x[core * TOK_PER_CORE:(core + 1) * TOK_PER_CORE]
        xh, xl = _pack_x_dr(xs)
        in_maps.append({"xh": xh, "xl": xl, "wh": wh, "wl": wl,
                        "ct": ct, "ac": ac, "bc": bc})
    return in_maps


def _assemble(results):
    parts = []
    for core in range(N_CORES):
        oc = results[core]["out"]  # [NT, 2, 128, 512] = [feat_tile, tok_half, feat, tok]
        parts.append(oc.transpose(1, 3, 0, 2).reshape(TOK_PER_CORE, OUT_F))
    out = np.concatenate(parts, axis=0)
    return out.reshape(4, 2048, OUT_F).astype(np.float32)


def kernel(input, weight, bias, centroids, assignments):
    from concourse.bass_utils import run_bass_kernel_spmd

    nc = _get_nc()
    in_maps = _prep_in_maps(input, weight, bias, centroids, assignments)
    res = run_bass_kernel_spmd(nc, in_maps, core_ids=list(range(N_CORES)))
    return _assemble(res.results)


# revision 5
# speedup vs baseline: 1.3514x; 1.3514x over previous
"""HKLinear (moe_routing) Trainium2 kernel — 8-core SPMD, data-parallel over
tokens, bf16 matmuls with a stall-free schedule.

Math (reference):
    x = input.reshape(n, in_f)                       n=8192, in_f=4096
    sm = softmax((x @ centroids.T) / T)              [n, 64], T=0.1
    hits = sm > 0.01
    query_sel = any(hits, axis=1)   -> provably ALL TRUE (max softmax >= 1/64 > 0.01)
    cluster_sel = any(hits, axis=0)                  [64]  (global over ALL tokens)
    row_sel = cluster_sel[assignments]               [out_f]
    out = (x @ W.T + b) * (query_sel & row_sel)      [n, out_f]

The PE floor on this box is 2048 matmuls x ~263 ns (512 cycles @2.4 GHz under
the observed 13/16 GPIO power throttle) ~= 539 us/core; everything else is
scheduled so the PE never waits:
  - DMA queue order: ct, W[0], W[1], ac, bc first, then x in 32 k-chunks, so
    feature-chunk 0/1 matmuls start ~4 us in and pace with the x stream.
  - PE order: chunk0, chunk1, routing, chunks 2..5, row-mask gather, chunks
    6..31.  The routing -> AllReduce(max) -> row-mask path (~40 us of latency)
    resolves while chunks 2..5 stream.
  - Chunks 0..5 park their unmasked psum in SBUF (frees the bank immediately)
    and get masked+biased once the collective lands; later chunks fuse
    mask+bias into the single DVE psum-drain op.
  - Redundant LDWEIGHTS (the two token-halves share each stationary W tile)
    are deduped post-legalization.
(fp8 DoubleRow was measured on this hw: a DR matmul costs the same 512 cycles
as bf16 for 2x the K-depth, so the >=2.4x extra passes needed to stay under
the 2e-2 error gate make every fp8 scheme a net loss.  bf16 measures 2.3e-3.)
"""

import numpy as np
import ml_dtypes

N_CORES = 8
IN_F = 4096
OUT_F = 4096
N_CLUSTERS = 64
THRESHOLD = 0.01
TEMPERATURE = 0.1
N_TOKENS = 8192
TOK_PER_CORE = N_TOKENS // N_CORES   # 1024

KT = IN_F // 128                     # 32 k-tiles
NT = OUT_F // 128                    # 32 out-feature tiles
MASK_AFTER = 6                       # chunks emitted before the mask gather
EXP_SHIFT = -30.0                    # softmax-invariant shift

BF16 = ml_dtypes.bfloat16


def _install_ldw_dedup():
    import concourse.tile as tile
    import concourse.mybir as mybir
    if getattr(tile, "_ldw_dedup_installed", False):
        return
    orig_legalize = tile.tile_legalize

    def ldw_key(i):
        return (str(i.ins[0]), str(i.perf_mode), str(i.tile_position),
                str(i.tile_size), str(i.is_transpose))

    def dedup_legalize(ordered, nc):
        out = orig_legalize(ordered, nc)
        for bb, insts in out.items():
            new, last_key, pending = [], None, None
            for i in insts:
                if getattr(i, "engine", None) != mybir.EngineType.PE:
                    new.append(i)
                    continue
                nm = type(i).__name__
                if nm == "InstLdweights":
                    k = ldw_key(i)
                    if k == last_key:
                        pending = i
                        continue
                    last_key = k
                    new.append(i)
                elif nm == "InstMatmult":
                    if pending is not None:
                        i.merge_dependencies_from(pending)
                        pending = None
                    new.append(i)
                else:
                    last_key, pending = None, None
                    new.append(i)
            out[bb] = new
        return out

    tile.tile_legalize = dedup_legalize
    tile._ldw_dedup_installed = True


def _build_bass():
    _install_ldw_dedup()
    import concourse.bass as bass
    import concourse.mybir as mybir
    import concourse.tile as tile
    from concourse import bacc
    from concourse.bass import ds

    f32 = mybir.dt.float32
    bf16 = mybir.dt.bfloat16

    nc = bacc.Bacc("TRN2", target_bir_lowering=False, debug=False,
                   num_devices=N_CORES)

    xk_d = nc.dram_tensor("xk", [128, KT, TOK_PER_CORE], bf16, kind="ExternalInput")
    wt_d = nc.dram_tensor("wt", [NT, 128, KT, 128], bf16, kind="ExternalInput")
    ct_d = nc.dram_tensor("ct", [128, KT, N_CLUSTERS], bf16, kind="ExternalInput")
    ac_d = nc.dram_tensor("ac", [N_CLUSTERS, NT, 128], bf16, kind="ExternalInput")
    bc_d = nc.dram_tensor("bc", [128, NT], f32, kind="ExternalInput")
    out_d = nc.dram_tensor("out", [NT, 2, 128, 512], f32, kind="ExternalOutput")

    with tile.TileContext(nc) as tc:
        with (
            tc.tile_pool(name="resident", bufs=1) as resident,
            tc.tile_pool(name="wpool", bufs=3) as wpool,
            tc.tile_pool(name="stage", bufs=2 * MASK_AFTER + 2) as stage,
            tc.tile_pool(name="opool", bufs=4) as opool,
            tc.tile_pool(name="route_sb", bufs=1) as route_sb,
            tc.tile_pool(name="psum_main", bufs=4, space="PSUM") as psum_main,
            tc.tile_pool(name="psum_l", bufs=2, space="PSUM") as psum_lp,
            tc.tile_pool(name="psum_scr", bufs=1, space="PSUM") as psum_scr,
            tc.tile_pool(name="cc_dram", bufs=1, space="DRAM") as cc_dram,
        ):
            # ---- DMA order: small/early operands first, then x k-chunks ----
            ct_sb = resident.tile([128, KT, N_CLUSTERS], bf16)
            nc.sync.dma_start(ct_sb[:], ct_d[:])
            w_first = []
            for n in range(2):
                w_sb = wpool.tile([128, KT, 128], bf16, tag="w_sb")
                nc.sync.dma_start(w_sb[:], wt_d[n])
                w_first.append(w_sb)
            a_sb = resident.tile([N_CLUSTERS, NT, 128], bf16)
            nc.sync.dma_start(a_sb[:], ac_d[:])
            bc_sb = resident.tile([128, NT], f32)
            nc.sync.dma_start(bc_sb[:], bc_d[:])
            xk_sb = resident.tile([128, KT, TOK_PER_CORE], bf16)
            for k in range(KT):
                nc.sync.dma_start(xk_sb[:, k], xk_d[:, k])

            shift_col = route_sb.tile([N_CLUSTERS, 1], f32)
            nc.vector.memset(shift_col[:], EXP_SHIFT)
            ones_c = route_sb.tile([N_CLUSTERS, 1], bf16)
            nc.vector.memset(ones_c[:], 1.0)
            ones_r = route_sb.tile([1, N_CLUSTERS], bf16)
            nc.vector.memset(ones_r[:], 1.0)

            # persistent mask tiles (written mid-program, after the CC)
            ms_sb = route_sb.tile([128, NT], f32)      # row mask (1.0/0.0)
            bmask_sb = route_sb.tile([128, NT], f32)   # mask * bias

            def emit_mm_block(n, w_sb=None):
                if w_sb is None:
                    w_sb = wpool.tile([128, KT, 128], bf16, tag="w_sb")
                    nc.sync.dma_start(w_sb[:], wt_d[n])
                pd = [psum_main.tile([128, 512], f32, tag="psum_d",
                                     name=f"psum_d_{n}_{h}") for h in range(2)]
                for k in range(KT):
                    for h in range(2):
                        nc.tensor.matmul(
                            pd[h][:], w_sb[:, k, :],
                            xk_sb[:, k, ds(h * 512, 512)],
                            start=(k == 0), stop=(k == KT - 1),
                        )
                return pd

            def emit_epilogue(n, h, src):
                o_sb = opool.tile([128, 512], f32, tag="o_sb")
                nc.vector.tensor_scalar(
                    o_sb[:], src,
                    ms_sb[:, ds(n, 1)], bmask_sb[:, ds(n, 1)],
                    op0=mybir.AluOpType.mult, op1=mybir.AluOpType.add,
                )
                nc.sync.dma_start(out_d[n, h], o_sb[:])

            def park(pd):
                st = []
                for h in range(2):
                    s_t = stage.tile([128, 512], f32, tag="stage")
                    nc.scalar.activation(s_t[:], pd[h][:],
                                         mybir.ActivationFunctionType.Copy)
                    st.append(s_t)
                return st

            parked = []
            # chunks 0/1 first: they pace with the x DMA stream
            parked.append(park(emit_mm_block(0, w_first[0])))
            parked.append(park(emit_mm_block(1, w_first[1])))

            # ---- routing: cluster logits, softmax threshold margin ----
            psum_ls = [psum_lp.tile([N_CLUSTERS, 512], f32, tag="psum_l",
                                    name=f"psum_l_{h}") for h in range(2)]
            for k in range(KT):
                for h in range(2):
                    nc.tensor.matmul(
                        psum_ls[h][:], ct_sb[:, k, :],
                        xk_sb[:, k, ds(h * 512, 512)],
                        start=(k == 0), stop=(k == KT - 1),
                    )
            cmax_h = []
            for h in range(2):
                e_f = route_sb.tile([N_CLUSTERS, 512], f32, tag="e_f", bufs=2)
                nc.scalar.activation(e_f[:], psum_ls[h][:],
                                     mybir.ActivationFunctionType.Exp,
                                     bias=shift_col[:], scale=1.0)
                e_b = route_sb.tile([N_CLUSTERS, 512], bf16, tag="e_b", bufs=2)
                nc.scalar.activation(e_b[:], psum_ls[h][:],
                                     mybir.ActivationFunctionType.Exp,
                                     bias=shift_col[:], scale=1.0)
                # S[t] = sum_c e (PE ones-reduction), then thr*S on 64 parts
                psum_s = psum_scr.tile([128, 512], f32, tag="scr")
                nc.tensor.matmul(psum_s[ds(0, 1), :], ones_c[:], e_b[:],
                                 start=True, stop=True)
                s_b = route_sb.tile([1, 512], bf16, tag="s_b", bufs=2)
                nc.scalar.activation(s_b[:], psum_s[ds(0, 1), :],
                                     mybir.ActivationFunctionType.Copy,
                                     scale=THRESHOLD)
                psum_b = psum_scr.tile([128, 512], f32, tag="scr")
                nc.tensor.matmul(psum_b[ds(0, N_CLUSTERS), :], ones_r[:], s_b[:],
                                 start=True, stop=True)
                d_sb = route_sb.tile([N_CLUSTERS, 512], f32, tag="d_sb", bufs=2)
                nc.vector.tensor_tensor(d_sb[:], e_f[:],
                                        psum_b[ds(0, N_CLUSTERS), :],
                                        op=mybir.AluOpType.subtract)
                cm = route_sb.tile([N_CLUSTERS, 1], f32, tag="cm", bufs=2)
                nc.vector.reduce_max(cm[:], d_sb[:], axis=mybir.AxisListType.X)
                cmax_h.append(cm)
            cmax = route_sb.tile([N_CLUSTERS, 1], f32)
            nc.vector.tensor_tensor(cmax[:], cmax_h[0][:], cmax_h[1][:],
                                    op=mybir.AluOpType.max)

            # ---- AllReduce(max) of the [64,1] margins across 8 cores ----
            cc_in = cc_dram.tile([N_CLUSTERS, 1], f32)
            cc_out = cc_dram.tile([N_CLUSTERS, 1], f32, addr_space="Shared")
            nc.gpsimd.dma_start(cc_in[:], cmax[:])
            nc.gpsimd.collective_compute(
                "AllReduce", mybir.AluOpType.max,
                replica_groups=[list(range(N_CORES))],
                ins=[cc_in.opt()], outs=[cc_out.opt()],
            )
            cmax_red = route_sb.tile([N_CLUSTERS, 1], f32)
            nc.gpsimd.dma_start(cmax_red[:], cc_out[:])
            sel_f = route_sb.tile([N_CLUSTERS, 1], f32)
            nc.vector.tensor_scalar(sel_f[:], cmax_red[:], 0.0, None,
                                    op0=mybir.AluOpType.is_gt)
            sel_bf = route_sb.tile([N_CLUSTERS, 1], bf16)
            nc.vector.tensor_copy(sel_bf[:], sel_f[:])

            # chunks 2..MASK_AFTER-1 stream while the collective resolves
            for n in range(2, MASK_AFTER):
                parked.append(park(emit_mm_block(n)))

            # ---- row mask from one-hot assignment columns (PE gather) ----
            psum_m = psum_scr.tile([128, 512], f32, tag="scr")
            for n in range(NT):
                nc.tensor.matmul(psum_m[:, ds(n, 1)], a_sb[:, n, :], sel_bf[:],
                                 start=True, stop=True)
            nc.scalar.activation(ms_sb[:], psum_m[:, ds(0, NT)],
                                 mybir.ActivationFunctionType.Copy)
            nc.vector.tensor_tensor(bmask_sb[:], ms_sb[:], bc_sb[:],
                                    op=mybir.AluOpType.mult)

            for n in range(MASK_AFTER):
                for h in range(2):
                    emit_epilogue(n, h, parked[n][h][:])

            for n in range(MASK_AFTER, NT):
                pd = emit_mm_block(n)
                for h in range(2):
                    emit_epilogue(n, h, pd[h][:])

    nc.compile()
    return nc


_NC_CACHE = None


def _get_nc():
    global _NC_CACHE
    if _NC_CACHE is None:
        _NC_CACHE = _build_bass()
    return _NC_CACHE


def _prep_in_maps(input, weight, bias, centroids, assignments):
    x = np.ascontiguousarray(np.asarray(input, dtype=np.float32).reshape(N_TOKENS, IN_F))
    w = np.asarray(weight, dtype=np.float32)
    b = np.asarray(bias, dtype=np.float32)
    c = np.asarray(centroids, dtype=np.float32)
    a = np.asarray(assignments)

    # wt[n, p, k, j] = W[n*128+j, k*128+p]
    wt = np.ascontiguousarray(
        w.T.reshape(KT, 128, NT, 128).transpose(2, 1, 0, 3)).astype(BF16)
    # ct[p, k, c] = centroids[c, k*128+p] / T
    ct = np.ascontiguousarray(
        (c / TEMPERATURE).T.reshape(KT, 128, N_CLUSTERS).transpose(1, 0, 2)).astype(BF16)
    ac = (a[None, :] == np.arange(N_CLUSTERS, dtype=a.dtype)[:, None])
    ac = np.ascontiguousarray(ac.reshape(N_CLUSTERS, NT, 128)).astype(BF16)
    bc = np.ascontiguousarray(b.reshape(NT, 128).T).astype(np.float32)

    in_maps = []
    for core in range(N_CORES):
        xs = x[core * TOK_PER_CORE:(core + 1) * TOK_PER_CORE]
        xk = np.ascontiguousarray(
            xs.T.reshape(KT, 128, TOK_PER_CORE).transpose(1, 0, 2)).astype(BF16)
        in_maps.append({"xk": xk, "wt": wt, "ct": ct, "ac": ac, "bc": bc})
    return in_maps


def _assemble(results):
    parts = []
    for core in range(N_CORES):
        oc = results[core]["out"]  # [NT, 2, 128, 512]
        parts.append(oc.transpose(1, 3, 0, 2).reshape(TOK_PER_CORE, OUT_F))
    out = np.concatenate(parts, axis=0)
    return out.reshape(4, 2048, OUT_F).astype(np.float32)


def kernel(input, weight, bias, centroids, assignments):
    from concourse.bass_utils import run_bass_kernel_spmd

    nc = _get_nc()
    in_maps = _prep_in_maps(input, weight, bias, centroids, assignments)
    res = run_bass_kernel_spmd(nc, in_maps, core_ids=list(range(N_CORES)))
    return _assemble(res.results)
